# revision 2
# baseline (speedup 1.0000x reference)
"""GroupMixAttention Trainium2 kernel (8-core SPMD, batch-parallel).

Problem: x[16,256,32,32]; per group g (4 groups of 64 ch):
  Q/K/V = wq/wk/wv[g] @ xg   (xg = [64, 1024])
  scores = (Q^T K)/8 ; attn = softmax(scores, -1) ; out = V @ attn^T
then y = wo @ concat(out).

Sharding: data-parallel over batch, 2 batches per core, no collectives.

Layout strategy per (batch, group-pair):
  - x2 [128, 1024] holds two groups' channels (natural slicing of x).
  - Q2/K2 [128, 1024] computed with row+col tiled matmuls (two 64-row
    groups concurrently on the PE array).
  - scoresT[m, n] = K^T Q computed chunk-wise (m in 8 chunks of 128) with
    the two groups packed in PE row-halves; exp on the scalar engine
    (scale=1/8 folded in; softmax max-subtraction skipped — scores are
    O(5) so fp32 exp is safe).
  - V^T chunks [128(m), 64(d)] computed directly (lhsT = x chunks) with a
    ones column appended -> PV matmul lhsT [128, 65]: row 64 of the
    accumulated output is the softmax denominator.
  - normalize: reciprocal (DVE) + partition_broadcast (gpsimd) + mul (DVE).
  - out_proj: wo^T chunks as lhsT over the stacked normalized heads.
"""

import os
import sys

import numpy as np

for _p in ("/opt/trn_rl_repo", "/root/.axon_site/_ro/trn_rl_repo"):
    if os.path.isdir(_p) and _p not in sys.path:
        sys.path.insert(0, _p)

import concourse.bass as bass
import concourse.mybir as mybir
import concourse.tile as tile
from concourse import bacc
from concourse.bass_utils import run_bass_kernel_spmd

F32 = mybir.dt.float32
EXP = mybir.ActivationFunctionType.Exp
N_CORES = 8
B_PER_CORE = 2  # 16 batches / 8 cores
NT = 1024  # H*W
GD = 64    # group dim
ts = bass.ts


def _build_program():
    nc = bacc.Bacc("TRN2", target_bir_lowering=False, debug=False,
                   num_devices=N_CORES)
    xs = nc.dram_tensor("xs", [B_PER_CORE, 2, 128, NT], F32,
                        kind="ExternalInput").ap()
    wqT = nc.dram_tensor("wqT", [2, 128, GD], F32, kind="ExternalInput").ap()
    wkT = nc.dram_tensor("wkT", [2, 128, GD], F32, kind="ExternalInput").ap()
    wvT = nc.dram_tensor("wvT", [2, 128, GD], F32, kind="ExternalInput").ap()
    woT = nc.dram_tensor("woT", [2, 128, 256], F32, kind="ExternalInput").ap()
    y = nc.dram_tensor("y", [B_PER_CORE, 256, NT], F32,
                       kind="ExternalOutput").ap()

    with tile.TileContext(nc) as tc:
        from contextlib import ExitStack
        with ExitStack() as ctx:
            const = ctx.enter_context(tc.tile_pool(name="const", bufs=1))
            xpool = ctx.enter_context(tc.tile_pool(name="xp", bufs=2))
            qk = ctx.enter_context(tc.tile_pool(name="qk", bufs=2))
            vtp = ctx.enter_context(tc.tile_pool(name="vt", bufs=2))
            ep = ctx.enter_context(tc.tile_pool(name="ep", bufs=3))
            sm = ctx.enter_context(tc.tile_pool(name="sm", bufs=2))
            onp = ctx.enter_context(tc.tile_pool(name="on", bufs=2))
            yp = ctx.enter_context(tc.tile_pool(name="yp", bufs=2))
            psS = ctx.enter_context(
                tc.tile_pool(name="psS", bufs=2, space="PSUM"))
            psAcc = ctx.enter_context(
                tc.tile_pool(name="psAcc", bufs=1, space="PSUM"))

            # Load weights once.
            w_sb = {}
            for name, dram in (("wq", wqT), ("wk", wkT), ("wv", wvT)):
                for p in range(2):
                    t = const.tile([128, GD], F32, tag=f"{name}{p}", name=f"{name}{p}")
                    nc.sync.dma_start(t[:], dram[p])
                    w_sb[name, p] = t
            wo_sb = []
            for k in range(2):
                t = const.tile([128, 256], F32, tag=f"wo{k}", name=f"wo{k}")
                nc.sync.dma_start(t[:], woT[k])
                wo_sb.append(t)

            for b in range(B_PER_CORE):
                outN = [onp.tile([128, NT], F32, tag=f"outN{p}", name=f"outN{p}")
                        for p in range(2)]
                for p in range(2):
                    x2 = xpool.tile([128, NT], F32, tag="x2")
                    nc.sync.dma_start(x2[:], xs[b, p])

                    # K2 / Q2 projections, both groups packed on the array.
                    K2 = qk.tile([128, NT], F32, tag="K2")
                    Q2 = qk.tile([128, NT], F32, tag="Q2")
                    for wname, dst in (("wk", K2), ("wq", Q2)):
                        ps = psAcc.tile([128, NT], F32, tag="big")
                        wt = w_sb[wname, p]
                        for nh in range(2):
                            s = ts(nh, 512)
                            nc.tensor.matmul(
                                ps[0:64, s], wt[0:64, :], x2[0:64, s],
                                start=True, stop=True, tile_position=(0, 0))
                            nc.tensor.matmul(
                                ps[64:128, s], wt[64:128, :], x2[64:128, s],
                                start=True, stop=True, tile_position=(64, 64))
                        nc.vector.tensor_copy(dst[:], ps[:])

                    # V^T chunks with ones column (denominator trick).
                    vts = [vtp.tile([128, 8 * (GD + 1)], F32, tag=f"vt{g}", name=f"vt{g}")
                           for g in range(2)]
                    for g in range(2):
                        nc.gpsimd.memset(vts[g][:], 1.0)
                    wv = w_sb["wv", p]
                    for mc in range(8):
                        pvA = psS.tile([128, GD], F32, tag="pss0")
                        pvB = psS.tile([128, GD], F32, tag="pss1")
                        nc.tensor.matmul(
                            pvA[:], x2[0:64, ts(mc, 128)], wv[0:64, :],
                            start=True, stop=True, tile_position=(0, 0))
                        nc.tensor.matmul(
                            pvB[:], x2[64:128, ts(mc, 128)], wv[64:128, :],
                            start=True, stop=True, tile_position=(64, 0))
                        c0 = 65 * mc
                        nc.vector.tensor_copy(vts[0][:, c0:c0 + GD], pvA[:])
                        nc.vector.tensor_copy(vts[1][:, c0:c0 + GD], pvB[:])

                    # Attention, n in two 512-halves to bound PSUM usage.
                    for nh in range(2):
                        ns = ts(nh, 512)
                        psO = [psAcc.tile([GD + 1, 512], F32, tag=f"psO{g}", name=f"psO{g}")
                               for g in range(2)]
                        for mc in range(8):
                            msl = ts(mc, 128)
                            pss = [psS.tile([128, 512], F32, tag=f"pss{g}", name=f"pss{g}")
                                   for g in range(2)]
                            nc.tensor.matmul(
                                pss[0][:], K2[0:64, msl], Q2[0:64, ns],
                                start=True, stop=True, tile_position=(0, 0))
                            nc.tensor.matmul(
                                pss[1][:], K2[64:128, msl], Q2[64:128, ns],
                                start=True, stop=True, tile_position=(64, 0))
                            for g in range(2):
                                E = ep.tile([128, 512], F32, tag=f"E{g}")
                                nc.scalar.activation(
                                    E[:], pss[g][:], EXP, scale=0.125)
                                c0 = 65 * mc
                                nc.tensor.matmul(
                                    psO[g][:], vts[g][:, c0:c0 + GD + 1],
                                    E[:], start=(mc == 0), stop=(mc == 7))
                        # softmax normalization + pack into outN
                        for g in range(2):
                            rec = sm.tile([1, 512], F32, tag="rec")
                            nc.vector.reciprocal(rec[:], psO[g][GD:GD + 1, :])
                            recb = sm.tile([GD, 512], F32, tag="recb")
                            nc.gpsimd.partition_broadcast(recb[:], rec[:])
                            nc.vector.tensor_mul(
                                outN[p][GD * g:GD * (g + 1), ns],
                                psO[g][0:GD, :], recb[:])

                # out_proj: y[b] = woT.T @ outN (contraction over C=256)
                for ec in range(2):
                    psY = psAcc.tile([128, NT], F32, tag="big")
                    for nh in range(2):
                        s = ts(nh, 512)
                        for kc in range(2):
                            nc.tensor.matmul(
                                psY[:, s], wo_sb[kc][:, ts(ec, 128)],
                                outN[kc][:, s],
                                start=(kc == 0), stop=(kc == 1))
                    yt = yp.tile([128, NT], F32, tag="yt")
                    nc.vector.tensor_copy(yt[:], psY[:])
                    nc.sync.dma_start(y[b][ts(ec, 128), :], yt[:])

    nc.finalize()
    return nc


_NC_CACHE = None


def _get_nc():
    global _NC_CACHE
    if _NC_CACHE is None:
        _NC_CACHE = _build_program()
    return _NC_CACHE


def _prep_inputs(x, wq, wk, wv, wo):
    B = x.shape[0]
    xr = np.ascontiguousarray(x.reshape(B, 2, 128, NT), dtype=np.float32)
    # [G, d, c] -> [G, c, d] -> [pair, 128, d]
    wqT = np.ascontiguousarray(
        wq.transpose(0, 2, 1).reshape(2, 128, GD), dtype=np.float32)
    wkT = np.ascontiguousarray(
        wk.transpose(0, 2, 1).reshape(2, 128, GD), dtype=np.float32)
    wvT = np.ascontiguousarray(
        wv.transpose(0, 2, 1).reshape(2, 128, GD), dtype=np.float32)
    woT = np.ascontiguousarray(wo.T.reshape(2, 128, 256), dtype=np.float32)
    return xr, wqT, wkT, wvT, woT


def run(x, wq, wk, wv, wo, trace=False, **trace_kwargs):
    x = np.asarray(x, dtype=np.float32)
    B, C, H, W = x.shape
    xr, wqT, wkT, wvT, woT = _prep_inputs(
        x, np.asarray(wq, np.float32), np.asarray(wk, np.float32),
        np.asarray(wv, np.float32), np.asarray(wo, np.float32))
    in_maps = []
    for c in range(N_CORES):
        in_maps.append({
            "xs": xr[c * B_PER_CORE:(c + 1) * B_PER_CORE],
            "wqT": wqT, "wkT": wkT, "wvT": wvT, "woT": woT,
        })
    res = run_bass_kernel_spmd(_get_nc(), in_maps, list(range(N_CORES)),
                               trace=trace, **trace_kwargs)
    outs = [res.results[c]["y"] for c in range(N_CORES)]
    yfull = np.concatenate(outs, axis=0).reshape(B, C, H, W)
    return yfull.astype(np.float32), res


def kernel(x, wq, wk, wv, wo):
    out, _ = run(x, wq, wk, wv, wo, trace=False)
    return out


# revision 10
# speedup vs baseline: 1.0979x; 1.0979x over previous
"""GroupMixAttention Trainium2 kernel (8-core SPMD, batch-parallel).

Problem: x[16,256,32,32]; per group g (4 groups of 64 ch):
  Q/K/V = wq/wk/wv[g] @ xg   (xg = [64, 1024])
  scores = (Q^T K)/8 ; attn = softmax(scores, -1) ; out = V @ attn^T
then y = wo @ concat(out).

Sharding: data-parallel over batch, 2 batches per core, no collectives.

Layout strategy per (batch, group-pair):
  - x2 [128, 1024] holds two groups' channels (natural slicing of x).
  - Q2/K2 [128, 1024] computed with row+col tiled matmuls (two 64-row
    groups concurrently on the PE array).
  - scoresT[m, n] = K^T Q computed chunk-wise (m in 8 chunks of 128) with
    the two groups packed in PE row-halves; exp on the scalar engine
    (scale=1/8 folded in; softmax max-subtraction skipped — scores are
    O(5) so fp32 exp is safe).
  - V^T chunks [128(m), 64(d)] computed directly (lhsT = x chunks) with a
    ones column appended -> PV matmul lhsT [128, 65]: row 64 of the
    accumulated output is the softmax denominator.
  - normalize: reciprocal (DVE) + partition_broadcast (gpsimd) + mul (DVE).
  - out_proj: wo^T chunks as lhsT over the stacked normalized heads.
"""

import os
import sys

import numpy as np

for _p in ("/opt/trn_rl_repo", "/root/.axon_site/_ro/trn_rl_repo"):
    if os.path.isdir(_p) and _p not in sys.path:
        sys.path.insert(0, _p)

import concourse.bass as bass
import concourse.mybir as mybir
import concourse.tile as tile
from concourse import bacc
from concourse.bass_utils import run_bass_kernel_spmd

F32 = mybir.dt.float32
EXP = mybir.ActivationFunctionType.Exp
N_CORES = 8
B_PER_CORE = 2  # 16 batches / 8 cores
NT = 1024  # H*W
GD = 64    # group dim
ts = bass.ts


def _build_program():
    nc = bacc.Bacc("TRN2", target_bir_lowering=False, debug=False,
                   num_devices=N_CORES)
    xs = nc.dram_tensor("xs", [B_PER_CORE, 2, 128, NT], F32,
                        kind="ExternalInput").ap()
    wqT = nc.dram_tensor("wqT", [2, 128, GD], F32, kind="ExternalInput").ap()
    wkT = nc.dram_tensor("wkT", [2, 128, GD], F32, kind="ExternalInput").ap()
    wvT = nc.dram_tensor("wvT", [2, 128, GD], F32, kind="ExternalInput").ap()
    woT = nc.dram_tensor("woT", [2, 128, 256], F32, kind="ExternalInput").ap()
    y = nc.dram_tensor("y", [B_PER_CORE, 256, NT], F32,
                       kind="ExternalOutput").ap()

    with tile.TileContext(nc) as tc:
        from contextlib import ExitStack
        with ExitStack() as ctx:
            const = ctx.enter_context(tc.tile_pool(name="const", bufs=1))
            xpool = ctx.enter_context(tc.tile_pool(name="xp", bufs=2))
            qk = ctx.enter_context(tc.tile_pool(name="qk", bufs=2))
            vtp = ctx.enter_context(tc.tile_pool(name="vt", bufs=2))
            ep = ctx.enter_context(tc.tile_pool(name="ep", bufs=3))
            sm = ctx.enter_context(tc.tile_pool(name="sm", bufs=2))
            onp = ctx.enter_context(tc.tile_pool(name="on", bufs=2))
            yp = ctx.enter_context(tc.tile_pool(name="yp", bufs=2))
            psS = ctx.enter_context(
                tc.tile_pool(name="psS", bufs=3, space="PSUM"))
            psAcc = ctx.enter_context(
                tc.tile_pool(name="psAcc", bufs=1, space="PSUM"))

            # Load weights once.
            w_sb = {}
            for name, dram in (("wq", wqT), ("wk", wkT), ("wv", wvT)):
                for p in range(2):
                    t = const.tile([128, GD], F32, tag=f"{name}{p}", name=f"{name}{p}")
                    nc.sync.dma_start(t[:], dram[p])
                    w_sb[name, p] = t
            ones128 = const.tile([128, 128], F32, tag="ones128",
                                 name="ones128")
            nc.gpsimd.memset(ones128[:], 1.0)
            wo_sb = []
            for k in range(2):
                t = const.tile([128, 256], F32, tag=f"wo{k}", name=f"wo{k}")
                nc.sync.dma_start(t[:], woT[k])
                wo_sb.append(t)

            for b in range(B_PER_CORE):
                outN = [onp.tile([128, NT], F32, tag=f"outN{p}", name=f"outN{p}")
                        for p in range(2)]
                for p in range(2):
                    x2 = xpool.tile([128, NT], F32, tag="x2")
                    nc.sync.dma_start(x2[:], xs[b, p])
                    den4 = sm.tile([128, 512], F32, tag="den4")

                    # K2 / Q2 projections, both groups packed on the array.
                    K2 = qk.tile([128, NT], F32, tag="K2")
                    Q2 = qk.tile([128, NT], F32, tag="Q2")
                    for wname, dst in (("wk", K2), ("wq", Q2)):
                        wt = w_sb[wname, p]
                        for nh in range(2):
                            s = ts(nh, 512)
                            ps = psS.tile([128, 512], F32, tag=f"pss{nh}",
                                          name=f"qkv{nh}")
                            nc.tensor.matmul(
                                ps[0:64, :], wt[0:64, :], x2[0:64, s],
                                start=True, stop=True, tile_position=(0, 0))
                            nc.tensor.matmul(
                                ps[64:128, :], wt[64:128, :], x2[64:128, s],
                                start=True, stop=True, tile_position=(64, 64))
                            nc.vector.tensor_copy(dst[:, s], ps[:])

                    # V^T chunks with ones column (denominator trick).
                    vts = [vtp.tile([128, 8 * (GD + 1)], F32, tag=f"vt{g}", name=f"vt{g}")
                           for g in range(2)]
                    for g in range(2):
                        nc.gpsimd.memset(vts[g][:], 1.0)
                    wv = w_sb["wv", p]
                    for mc in range(8):
                        pvA = psS.tile([128, GD], F32, tag="pss0")
                        pvB = psS.tile([128, GD], F32, tag="pss1")
                        nc.tensor.matmul(
                            pvA[:], x2[0:64, ts(mc, 128)], wv[0:64, :],
                            start=True, stop=True, tile_position=(0, 0))
                        nc.tensor.matmul(
                            pvB[:], x2[64:128, ts(mc, 128)], wv[64:128, :],
                            start=True, stop=True, tile_position=(64, 0))
                        c0 = 65 * mc
                        nc.vector.tensor_copy(vts[0][:, c0:c0 + GD], pvA[:])
                        nc.vector.tensor_copy(vts[1][:, c0:c0 + GD], pvB[:])

                    # Attention, n in two 512-halves to bound PSUM usage.
                    for nh in range(2):
                        ns = ts(nh, 512)
                        psO = [psAcc.tile([GD + 1, 512], F32, tag=f"psO{g}", name=f"psO{g}")
                               for g in range(2)]
                        for mc in range(8):
                            msl = ts(mc, 128)
                            pss = [psS.tile([128, 512], F32, tag=f"pss{g}", name=f"pss{g}")
                                   for g in range(2)]
                            nc.tensor.matmul(
                                pss[0][:], K2[0:64, msl], Q2[0:64, ns],
                                start=True, stop=True, tile_position=(0, 0))
                            nc.tensor.matmul(
                                pss[1][:], K2[64:128, msl], Q2[64:128, ns],
                                start=True, stop=True, tile_position=(64, 0))
                            for g in range(2):
                                E = ep.tile([128, 512], F32, tag=f"E{g}")
                                nc.scalar.activation(
                                    E[:], pss[g][:], EXP, scale=0.125)
                                c0 = 65 * mc
                                nc.tensor.matmul(
                                    psO[g][:], vts[g][:, c0:c0 + GD + 1],
                                    E[:], start=(mc == 0), stop=(mc == 7))
                        # stage numerators + denominators; normalize later
                        # (keeps the slow single-partition reciprocal off
                        # the PE critical path and frees psO banks early)
                        for g in range(2):
                            nc.vector.tensor_copy(
                                outN[p][GD * g:GD * (g + 1), ns],
                                psO[g][0:GD, :])
                            r = 32 * (2 * nh + g)
                            nc.vector.tensor_copy(
                                den4[r:r + 1, :], psO[g][GD:GD + 1, :])

                    # one batched reciprocal for all 4 (nh, g) denom rows
                    rec4 = sm.tile([128, 512], F32, tag="rec4")
                    nc.vector.reciprocal(rec4[:], den4[:])
                    for nh in range(2):
                        ns = ts(nh, 512)
                        psR = psS.tile([128, 512], F32, tag=f"pss{nh}",
                                       name=f"psR{nh}")
                        for g in range(2):
                            r = 32 * (2 * nh + g)
                            nc.tensor.matmul(
                                psR[GD * g:GD * (g + 1), :],
                                ones128[r:r + 1, 0:GD], rec4[r:r + 1, :],
                                start=True, stop=True,
                                tile_position=(r, GD * g))
                            rows = outN[p][GD * g:GD * (g + 1), ns]
                            nc.vector.tensor_mul(
                                rows, rows, psR[GD * g:GD * (g + 1), :])

                # out_proj: y[b] = woT.T @ outN (contraction over C=256)
                for ec in range(2):
                    yt = yp.tile([128, NT], F32, tag="yt")
                    for nh in range(2):
                        s = ts(nh, 512)
                        psY = psS.tile([128, 512], F32, tag=f"pss{nh}",
                                       name=f"psY{nh}")
                        for kc in range(2):
                            nc.tensor.matmul(
                                psY[:], wo_sb[kc][:, ts(ec, 128)],
                                outN[kc][:, s],
                                start=(kc == 0), stop=(kc == 1))
                        nc.vector.tensor_copy(yt[:, s], psY[:])
                    nc.sync.dma_start(y[b][ts(ec, 128), :], yt[:])

    nc.finalize()
    return nc


_NC_CACHE = None


def _get_nc():
    global _NC_CACHE
    if _NC_CACHE is None:
        _NC_CACHE = _build_program()
    return _NC_CACHE


def _prep_inputs(x, wq, wk, wv, wo):
    B = x.shape[0]
    xr = np.ascontiguousarray(x.reshape(B, 2, 128, NT), dtype=np.float32)
    # [G, d, c] -> [G, c, d] -> [pair, 128, d]
    wqT = np.ascontiguousarray(
        wq.transpose(0, 2, 1).reshape(2, 128, GD), dtype=np.float32)
    wkT = np.ascontiguousarray(
        wk.transpose(0, 2, 1).reshape(2, 128, GD), dtype=np.float32)
    wvT = np.ascontiguousarray(
        wv.transpose(0, 2, 1).reshape(2, 128, GD), dtype=np.float32)
    woT = np.ascontiguousarray(wo.T.reshape(2, 128, 256), dtype=np.float32)
    return xr, wqT, wkT, wvT, woT


def run(x, wq, wk, wv, wo, trace=False, **trace_kwargs):
    x = np.asarray(x, dtype=np.float32)
    B, C, H, W = x.shape
    xr, wqT, wkT, wvT, woT = _prep_inputs(
        x, np.asarray(wq, np.float32), np.asarray(wk, np.float32),
        np.asarray(wv, np.float32), np.asarray(wo, np.float32))
    in_maps = []
    for c in range(N_CORES):
        in_maps.append({
            "xs": xr[c * B_PER_CORE:(c + 1) * B_PER_CORE],
            "wqT": wqT, "wkT": wkT, "wvT": wvT, "woT": woT,
        })
    res = run_bass_kernel_spmd(_get_nc(), in_maps, list(range(N_CORES)),
                               trace=trace, **trace_kwargs)
    outs = [res.results[c]["y"] for c in range(N_CORES)]
    yfull = np.concatenate(outs, axis=0).reshape(B, C, H, W)
    return yfull.astype(np.float32), res


def kernel(x, wq, wk, wv, wo):
    out, _ = run(x, wq, wk, wv, wo, trace=False)
    return out


# revision 11
# speedup vs baseline: 1.3693x; 1.2471x over previous
"""GroupMixAttention Trainium2 kernel (8-core SPMD, batch-parallel).

Problem: x[16,256,32,32]; per group g (4 groups of 64 ch):
  Q/K/V = wq/wk/wv[g] @ xg   (xg = [64, 1024])
  scores = (Q^T K)/8 ; attn = softmax(scores, -1) ; out = V @ attn^T
then y = wo @ concat(out).

Sharding: data-parallel over batch, 2 batches per core, no collectives.

Layout strategy per (batch, group-pair):
  - x2 [128, 1024] holds two groups' channels (natural slicing of x).
  - Q2/K2 [128, 1024] computed with row+col tiled matmuls (two 64-row
    groups concurrently on the PE array).
  - scoresT[m, n] = K^T Q computed chunk-wise (m in 8 chunks of 128) with
    the two groups packed in PE row-halves; exp on the scalar engine
    (scale=1/8 folded in; softmax max-subtraction skipped — scores are
    O(5) so fp32 exp is safe).
  - V^T chunks [128(m), 64(d)] computed directly (lhsT = x chunks) with a
    ones column appended -> PV matmul lhsT [128, 65]: row 64 of the
    accumulated output is the softmax denominator.
  - normalize: reciprocal (DVE) + partition_broadcast (gpsimd) + mul (DVE).
  - out_proj: wo^T chunks as lhsT over the stacked normalized heads.
"""

import os
import sys

import numpy as np

for _p in ("/opt/trn_rl_repo", "/root/.axon_site/_ro/trn_rl_repo"):
    if os.path.isdir(_p) and _p not in sys.path:
        sys.path.insert(0, _p)

import concourse.bass as bass
import concourse.mybir as mybir
import concourse.tile as tile
from concourse import bacc
from concourse.bass_utils import run_bass_kernel_spmd

F32 = mybir.dt.float32
EXP = mybir.ActivationFunctionType.Exp
N_CORES = 8
B_PER_CORE = 2  # 16 batches / 8 cores
NT = 1024  # H*W
GD = 64    # group dim
ts = bass.ts


def _build_program():
    nc = bacc.Bacc("TRN2", target_bir_lowering=False, debug=False,
                   num_devices=N_CORES)
    xs = nc.dram_tensor("xs", [B_PER_CORE, 2, 128, NT], F32,
                        kind="ExternalInput").ap()
    wqT = nc.dram_tensor("wqT", [2, 128, GD], F32, kind="ExternalInput").ap()
    wkT = nc.dram_tensor("wkT", [2, 128, GD], F32, kind="ExternalInput").ap()
    wvT = nc.dram_tensor("wvT", [2, 128, GD], F32, kind="ExternalInput").ap()
    woT = nc.dram_tensor("woT", [2, 128, 256], F32, kind="ExternalInput").ap()
    y = nc.dram_tensor("y", [B_PER_CORE, 256, NT], F32,
                       kind="ExternalOutput").ap()

    with tile.TileContext(nc) as tc:
        from contextlib import ExitStack
        with ExitStack() as ctx:
            const = ctx.enter_context(tc.tile_pool(name="const", bufs=1))
            xpool = ctx.enter_context(tc.tile_pool(name="xp", bufs=2))
            qk = ctx.enter_context(tc.tile_pool(name="qk", bufs=2))
            vtp = ctx.enter_context(tc.tile_pool(name="vt", bufs=2))
            ep = ctx.enter_context(tc.tile_pool(name="ep", bufs=3))
            sm = ctx.enter_context(tc.tile_pool(name="sm", bufs=2))
            onp = ctx.enter_context(tc.tile_pool(name="on", bufs=2))
            yp = ctx.enter_context(tc.tile_pool(name="yp", bufs=2))
            psS = ctx.enter_context(
                tc.tile_pool(name="psS", bufs=3, space="PSUM"))
            psAcc = ctx.enter_context(
                tc.tile_pool(name="psAcc", bufs=1, space="PSUM"))

            # Load weights once.
            w_sb = {}
            for name, dram in (("wq", wqT), ("wk", wkT), ("wv", wvT)):
                for p in range(2):
                    t = const.tile([128, GD], F32, tag=f"{name}{p}", name=f"{name}{p}")
                    nc.sync.dma_start(t[:], dram[p])
                    w_sb[name, p] = t
            ones128 = const.tile([128, 128], F32, tag="ones128",
                                 name="ones128")
            nc.gpsimd.memset(ones128[:], 1.0)
            wo_sb = []
            for k in range(2):
                t = const.tile([128, 256], F32, tag=f"wo{k}", name=f"wo{k}")
                nc.sync.dma_start(t[:], woT[k])
                wo_sb.append(t)

            for b in range(B_PER_CORE):
                outN = [onp.tile([128, NT], F32, tag=f"outN{p}", name=f"outN{p}")
                        for p in range(2)]
                norm_jobs = []
                for p in range(2):
                    x2 = xpool.tile([128, NT], F32, tag="x2")
                    nc.sync.dma_start(x2[:], xs[b, p])
                    den4 = sm.tile([128, 512], F32, tag="den4")

                    # K2 / Q2 projections, both groups packed on the array.
                    K2 = qk.tile([128, NT], F32, tag="K2")
                    Q2 = qk.tile([128, NT], F32, tag="Q2")
                    for wname, dst in (("wk", K2), ("wq", Q2)):
                        wt = w_sb[wname, p]
                        for nh in range(2):
                            s = ts(nh, 512)
                            ps = psS.tile([128, 512], F32, tag=f"pss{nh}",
                                          name=f"qkv{nh}")
                            nc.tensor.matmul(
                                ps[0:64, :], wt[0:64, :], x2[0:64, s],
                                start=True, stop=True, tile_position=(0, 0))
                            nc.tensor.matmul(
                                ps[64:128, :], wt[64:128, :], x2[64:128, s],
                                start=True, stop=True, tile_position=(64, 64))
                            nc.vector.tensor_copy(dst[:, s], ps[:])

                    # V^T chunks with ones column (denominator trick).
                    vts = [vtp.tile([128, 8 * (GD + 1)], F32, tag=f"vt{g}", name=f"vt{g}")
                           for g in range(2)]
                    for g in range(2):
                        nc.gpsimd.memset(vts[g][:], 1.0)
                    wv = w_sb["wv", p]
                    for mc in range(8):
                        pvA = psS.tile([128, GD], F32, tag="pss0")
                        pvB = psS.tile([128, GD], F32, tag="pss1")
                        nc.tensor.matmul(
                            pvA[:], x2[0:64, ts(mc, 128)], wv[0:64, :],
                            start=True, stop=True, tile_position=(0, 0))
                        nc.tensor.matmul(
                            pvB[:], x2[64:128, ts(mc, 128)], wv[64:128, :],
                            start=True, stop=True, tile_position=(64, 0))
                        c0 = 65 * mc
                        nc.vector.tensor_copy(vts[0][:, c0:c0 + GD], pvA[:])
                        nc.vector.tensor_copy(vts[1][:, c0:c0 + GD], pvB[:])

                    # Attention, n in two 512-halves to bound PSUM usage.
                    for nh in range(2):
                        ns = ts(nh, 512)
                        psO = [psAcc.tile([GD + 1, 512], F32, tag=f"psO{g}", name=f"psO{g}")
                               for g in range(2)]
                        sc = {}
                        for step in range(10):
                            if step < 8:
                                msl = ts(step, 128)
                                pss = [psS.tile([128, 512], F32,
                                                tag=f"pss{g}",
                                                name=f"pss{g}_{step}")
                                       for g in range(2)]
                                nc.tensor.matmul(
                                    pss[0][:], K2[0:64, msl], Q2[0:64, ns],
                                    start=True, stop=True,
                                    tile_position=(0, 0))
                                nc.tensor.matmul(
                                    pss[1][:], K2[64:128, msl],
                                    Q2[64:128, ns],
                                    start=True, stop=True,
                                    tile_position=(64, 0))
                                sc[step] = pss
                            if step >= 2:
                                mc = step - 2
                                for g in range(2):
                                    E = ep.tile([128, 512], F32, tag=f"E{g}",
                                                name=f"E{g}_{mc}")
                                    nc.scalar.activation(
                                        E[:], sc[mc][g][:], EXP, scale=0.125)
                                    c0 = 65 * mc
                                    nc.tensor.matmul(
                                        psO[g][:], vts[g][:, c0:c0 + GD + 1],
                                        E[:], start=(mc == 0), stop=(mc == 7))
                        # stage numerators + denominators; normalize later
                        # (keeps the slow single-partition reciprocal off
                        # the PE critical path and frees psO banks early)
                        for g in range(2):
                            nc.vector.tensor_copy(
                                outN[p][GD * g:GD * (g + 1), ns],
                                psO[g][0:GD, :])
                            r = 32 * (2 * nh + g)
                            nc.vector.tensor_copy(
                                den4[r:r + 1, :], psO[g][GD:GD + 1, :])

                    # batched reciprocal now (DVE, overlaps next pair);
                    # broadcast+multiply deferred to batch end
                    rec4 = sm.tile([128, 512], F32, tag="rec4",
                                   name=f"rec4_{p}")
                    nc.vector.reciprocal(rec4[:], den4[:])
                    norm_jobs.append((p, rec4))

                for p, rec4 in norm_jobs:
                    for nh in range(2):
                        ns = ts(nh, 512)
                        psR = psS.tile([128, 512], F32, tag=f"pss{nh}",
                                       name=f"psR{nh}_{p}")
                        for g in range(2):
                            r = 32 * (2 * nh + g)
                            nc.tensor.matmul(
                                psR[GD * g:GD * (g + 1), :],
                                ones128[r:r + 1, 0:GD], rec4[r:r + 1, :],
                                start=True, stop=True,
                                tile_position=(r, GD * g))
                            rows = outN[p][GD * g:GD * (g + 1), ns]
                            nc.vector.tensor_mul(
                                rows, rows, psR[GD * g:GD * (g + 1), :])

                # out_proj: y[b] = woT.T @ outN (contraction over C=256)
                for ec in range(2):
                    yt = yp.tile([128, NT], F32, tag="yt")
                    for nh in range(2):
                        s = ts(nh, 512)
                        psY = psS.tile([128, 512], F32, tag=f"pss{nh}",
                                       name=f"psY{nh}")
                        for kc in range(2):
                            nc.tensor.matmul(
                                psY[:], wo_sb[kc][:, ts(ec, 128)],
                                outN[kc][:, s],
                                start=(kc == 0), stop=(kc == 1))
                        nc.vector.tensor_copy(yt[:, s], psY[:])
                    nc.sync.dma_start(y[b][ts(ec, 128), :], yt[:])

    nc.finalize()
    return nc


_NC_CACHE = None


def _get_nc():
    global _NC_CACHE
    if _NC_CACHE is None:
        _NC_CACHE = _build_program()
    return _NC_CACHE


def _prep_inputs(x, wq, wk, wv, wo):
    B = x.shape[0]
    xr = np.ascontiguousarray(x.reshape(B, 2, 128, NT), dtype=np.float32)
    # [G, d, c] -> [G, c, d] -> [pair, 128, d]
    wqT = np.ascontiguousarray(
        wq.transpose(0, 2, 1).reshape(2, 128, GD), dtype=np.float32)
    wkT = np.ascontiguousarray(
        wk.transpose(0, 2, 1).reshape(2, 128, GD), dtype=np.float32)
    wvT = np.ascontiguousarray(
        wv.transpose(0, 2, 1).reshape(2, 128, GD), dtype=np.float32)
    woT = np.ascontiguousarray(wo.T.reshape(2, 128, 256), dtype=np.float32)
    return xr, wqT, wkT, wvT, woT


def run(x, wq, wk, wv, wo, trace=False, **trace_kwargs):
    x = np.asarray(x, dtype=np.float32)
    B, C, H, W = x.shape
    xr, wqT, wkT, wvT, woT = _prep_inputs(
        x, np.asarray(wq, np.float32), np.asarray(wk, np.float32),
        np.asarray(wv, np.float32), np.asarray(wo, np.float32))
    in_maps = []
    for c in range(N_CORES):
        in_maps.append({
            "xs": xr[c * B_PER_CORE:(c + 1) * B_PER_CORE],
            "wqT": wqT, "wkT": wkT, "wvT": wvT, "woT": woT,
        })
    res = run_bass_kernel_spmd(_get_nc(), in_maps, list(range(N_CORES)),
                               trace=trace, **trace_kwargs)
    outs = [res.results[c]["y"] for c in range(N_CORES)]
    yfull = np.concatenate(outs, axis=0).reshape(B, C, H, W)
    return yfull.astype(np.float32), res


def kernel(x, wq, wk, wv, wo):
    out, _ = run(x, wq, wk, wv, wo, trace=False)
    return out


# revision 13
# speedup vs baseline: 1.8518x; 1.3524x over previous
"""GroupMixAttention Trainium2 kernel (8-core SPMD, batch-parallel).

Problem: x[16,256,32,32]; per group g (4 groups of 64 ch):
  Q/K/V = wq/wk/wv[g] @ xg   (xg = [64, 1024])
  scores = (Q^T K)/8 ; attn = softmax(scores, -1) ; out = V @ attn^T
then y = wo @ concat(out).

Sharding: data-parallel over batch, 2 batches per core, no collectives.

Layout strategy per (batch, group-pair):
  - x2 [128, 1024] holds two groups' channels (natural slicing of x).
  - Q2/K2 [128, 1024] computed with row+col tiled matmuls (two 64-row
    groups concurrently on the PE array).
  - scoresT[m, n] = K^T Q computed chunk-wise (m in 8 chunks of 128) with
    the two groups packed in PE row-halves; exp on the scalar engine
    (scale=1/8 folded in; softmax max-subtraction skipped — scores are
    O(5) so fp32 exp is safe).
  - V^T chunks [128(m), 64(d)] computed directly (lhsT = x chunks) with a
    ones column appended -> PV matmul lhsT [128, 65]: row 64 of the
    accumulated output is the softmax denominator.
  - normalize: reciprocal (DVE) + partition_broadcast (gpsimd) + mul (DVE).
  - out_proj: wo^T chunks as lhsT over the stacked normalized heads.
"""

import os
import sys

import numpy as np

for _p in ("/opt/trn_rl_repo", "/root/.axon_site/_ro/trn_rl_repo"):
    if os.path.isdir(_p) and _p not in sys.path:
        sys.path.insert(0, _p)

import concourse.bass as bass
import concourse.mybir as mybir
import concourse.tile as tile
from concourse import bacc
from concourse.bass_utils import run_bass_kernel_spmd

F32 = mybir.dt.float32
BF16 = mybir.dt.bfloat16
EXP = mybir.ActivationFunctionType.Exp
N_CORES = 8
B_PER_CORE = 2  # 16 batches / 8 cores
NT = 1024  # H*W
GD = 64    # group dim
ts = bass.ts


def _build_program():
    nc = bacc.Bacc("TRN2", target_bir_lowering=False, debug=False,
                   num_devices=N_CORES)
    xs = nc.dram_tensor("xs", [B_PER_CORE, 2, 128, NT], F32,
                        kind="ExternalInput").ap()
    wqT = nc.dram_tensor("wqT", [2, 128, GD], F32, kind="ExternalInput").ap()
    wkT = nc.dram_tensor("wkT", [2, 128, GD], F32, kind="ExternalInput").ap()
    wvT = nc.dram_tensor("wvT", [2, 128, GD], F32, kind="ExternalInput").ap()
    woT = nc.dram_tensor("woT", [2, 128, 256], F32, kind="ExternalInput").ap()
    y = nc.dram_tensor("y", [B_PER_CORE, 256, NT], F32,
                       kind="ExternalOutput").ap()

    with tile.TileContext(nc) as tc:
        from contextlib import ExitStack
        with ExitStack() as ctx:
            const = ctx.enter_context(tc.tile_pool(name="const", bufs=1))
            xpool = ctx.enter_context(tc.tile_pool(name="xp", bufs=2))
            qk = ctx.enter_context(tc.tile_pool(name="qk", bufs=2))
            vtp = ctx.enter_context(tc.tile_pool(name="vt", bufs=2))
            ep = ctx.enter_context(tc.tile_pool(name="ep", bufs=3))
            sm = ctx.enter_context(tc.tile_pool(name="sm", bufs=2))
            onp = ctx.enter_context(tc.tile_pool(name="on", bufs=2))
            yp = ctx.enter_context(tc.tile_pool(name="yp", bufs=2))
            psS = ctx.enter_context(
                tc.tile_pool(name="psS", bufs=3, space="PSUM"))
            psAcc = ctx.enter_context(
                tc.tile_pool(name="psAcc", bufs=1, space="PSUM"))

            # Load weights once.
            w_sb = {}
            for name, dram in (("wq", wqT), ("wk", wkT), ("wv", wvT)):
                for p in range(2):
                    t = const.tile([128, GD], F32, tag=f"{name}{p}", name=f"{name}{p}")
                    nc.sync.dma_start(t[:], dram[p])
                    w_sb[name, p] = t
            ones128 = const.tile([128, 128], F32, tag="ones128",
                                 name="ones128")
            nc.gpsimd.memset(ones128[:], 1.0)
            wo_sb = []
            for k in range(2):
                t = const.tile([128, 256], F32, tag=f"wo{k}", name=f"wo{k}")
                nc.sync.dma_start(t[:], woT[k])
                wo_sb.append(t)

            for b in range(B_PER_CORE):
                outN = [onp.tile([128, NT], F32, tag=f"outN{p}", name=f"outN{p}")
                        for p in range(2)]
                norm_jobs = []
                for p in range(2):
                    x2 = xpool.tile([128, NT], F32, tag="x2")
                    nc.sync.dma_start(x2[:], xs[b, p])
                    den4 = sm.tile([128, 512], F32, tag="den4")

                    # K2 / Q2 projections, both groups packed on the array.
                    K2 = qk.tile([128, NT], F32, tag="K2")
                    Q2 = qk.tile([128, NT], F32, tag="Q2")
                    for wname, dst in (("wk", K2), ("wq", Q2)):
                        wt = w_sb[wname, p]
                        for nh in range(2):
                            s = ts(nh, 512)
                            ps = psS.tile([128, 512], F32, tag=f"pss{nh}",
                                          name=f"qkv{nh}")
                            nc.tensor.matmul(
                                ps[0:64, :], wt[0:64, :], x2[0:64, s],
                                start=True, stop=True, tile_position=(0, 0))
                            nc.tensor.matmul(
                                ps[64:128, :], wt[64:128, :], x2[64:128, s],
                                start=True, stop=True, tile_position=(64, 64))
                            nc.vector.tensor_copy(dst[:, s], ps[:])

                    # V^T chunks with ones column (denominator trick).
                    vts = [vtp.tile([128, 8 * (GD + 1)], BF16, tag=f"vt{g}", name=f"vt{g}")
                           for g in range(2)]
                    for g in range(2):
                        nc.vector.memset(vts[g][:], 1.0)
                    wv = w_sb["wv", p]
                    for mc in range(8):
                        pvA = psS.tile([128, GD], F32, tag="pss0")
                        pvB = psS.tile([128, GD], F32, tag="pss1")
                        nc.tensor.matmul(
                            pvA[:], x2[0:64, ts(mc, 128)], wv[0:64, :],
                            start=True, stop=True, tile_position=(0, 0))
                        nc.tensor.matmul(
                            pvB[:], x2[64:128, ts(mc, 128)], wv[64:128, :],
                            start=True, stop=True, tile_position=(64, 0))
                        c0 = 65 * mc
                        nc.vector.tensor_copy(vts[0][:, c0:c0 + GD], pvA[:])
                        nc.vector.tensor_copy(vts[1][:, c0:c0 + GD], pvB[:])

                    # Attention, n in two 512-halves to bound PSUM usage.
                    for nh in range(2):
                        ns = ts(nh, 512)
                        psO = [psAcc.tile([GD + 1, 512], F32, tag=f"psO{g}", name=f"psO{g}")
                               for g in range(2)]
                        sc = {}
                        for step in range(10):
                            if step < 8:
                                msl = ts(step, 128)
                                pss = [psS.tile([128, 512], F32,
                                                tag=f"pss{g}",
                                                name=f"pss{g}_{step}")
                                       for g in range(2)]
                                nc.tensor.matmul(
                                    pss[0][:], K2[0:64, msl], Q2[0:64, ns],
                                    start=True, stop=True,
                                    tile_position=(0, 0))
                                nc.tensor.matmul(
                                    pss[1][:], K2[64:128, msl],
                                    Q2[64:128, ns],
                                    start=True, stop=True,
                                    tile_position=(64, 0))
                                sc[step] = pss
                            if step >= 2:
                                mc = step - 2
                                for g in range(2):
                                    E = ep.tile([128, 512], BF16, tag=f"E{g}",
                                                name=f"E{g}_{mc}")
                                    nc.scalar.activation(
                                        E[:], sc[mc][g][:], EXP, scale=0.125)
                                    c0 = 65 * mc
                                    nc.tensor.matmul(
                                        psO[g][:], vts[g][:, c0:c0 + GD + 1],
                                        E[:], start=(mc == 0), stop=(mc == 7))
                        # stage numerators + denominators; normalize later
                        # (keeps the slow single-partition reciprocal off
                        # the PE critical path and frees psO banks early)
                        for g in range(2):
                            nc.vector.tensor_copy(
                                outN[p][GD * g:GD * (g + 1), ns],
                                psO[g][0:GD, :])
                            r = 32 * (2 * nh + g)
                            nc.vector.tensor_copy(
                                den4[r:r + 1, :], psO[g][GD:GD + 1, :])

                    # batched reciprocal now (DVE, overlaps next pair);
                    # broadcast+multiply deferred to batch end
                    rec4 = sm.tile([128, 512], F32, tag="rec4",
                                   name=f"rec4_{p}")
                    nc.vector.reciprocal(rec4[:], den4[:])
                    norm_jobs.append((p, rec4))

                for p, rec4 in norm_jobs:
                    for nh in range(2):
                        ns = ts(nh, 512)
                        psR = psS.tile([128, 512], F32, tag=f"pss{nh}",
                                       name=f"psR{nh}_{p}")
                        for g in range(2):
                            r = 32 * (2 * nh + g)
                            nc.tensor.matmul(
                                psR[GD * g:GD * (g + 1), :],
                                ones128[r:r + 1, 0:GD], rec4[r:r + 1, :],
                                start=True, stop=True,
                                tile_position=(r, GD * g))
                            rows = outN[p][GD * g:GD * (g + 1), ns]
                            nc.vector.tensor_mul(
                                rows, rows, psR[GD * g:GD * (g + 1), :])

                # out_proj: y[b] = woT.T @ outN (contraction over C=256)
                for ec in range(2):
                    yt = yp.tile([128, NT], F32, tag="yt")
                    for nh in range(2):
                        s = ts(nh, 512)
                        psY = psS.tile([128, 512], F32, tag=f"pss{nh}",
                                       name=f"psY{nh}")
                        for kc in range(2):
                            nc.tensor.matmul(
                                psY[:], wo_sb[kc][:, ts(ec, 128)],
                                outN[kc][:, s],
                                start=(kc == 0), stop=(kc == 1))
                        nc.vector.tensor_copy(yt[:, s], psY[:])
                    nc.sync.dma_start(y[b][ts(ec, 128), :], yt[:])

    nc.finalize()
    return nc


_NC_CACHE = None


def _get_nc():
    global _NC_CACHE
    if _NC_CACHE is None:
        _NC_CACHE = _build_program()
    return _NC_CACHE


def _prep_inputs(x, wq, wk, wv, wo):
    B = x.shape[0]
    xr = np.ascontiguousarray(x.reshape(B, 2, 128, NT), dtype=np.float32)
    # [G, d, c] -> [G, c, d] -> [pair, 128, d]
    wqT = np.ascontiguousarray(
        wq.transpose(0, 2, 1).reshape(2, 128, GD), dtype=np.float32)
    wkT = np.ascontiguousarray(
        wk.transpose(0, 2, 1).reshape(2, 128, GD), dtype=np.float32)
    wvT = np.ascontiguousarray(
        wv.transpose(0, 2, 1).reshape(2, 128, GD), dtype=np.float32)
    woT = np.ascontiguousarray(wo.T.reshape(2, 128, 256), dtype=np.float32)
    return xr, wqT, wkT, wvT, woT


def run(x, wq, wk, wv, wo, trace=False, **trace_kwargs):
    x = np.asarray(x, dtype=np.float32)
    B, C, H, W = x.shape
    xr, wqT, wkT, wvT, woT = _prep_inputs(
        x, np.asarray(wq, np.float32), np.asarray(wk, np.float32),
        np.asarray(wv, np.float32), np.asarray(wo, np.float32))
    in_maps = []
    for c in range(N_CORES):
        in_maps.append({
            "xs": xr[c * B_PER_CORE:(c + 1) * B_PER_CORE],
            "wqT": wqT, "wkT": wkT, "wvT": wvT, "woT": woT,
        })
    res = run_bass_kernel_spmd(_get_nc(), in_maps, list(range(N_CORES)),
                               trace=trace, **trace_kwargs)
    outs = [res.results[c]["y"] for c in range(N_CORES)]
    yfull = np.concatenate(outs, axis=0).reshape(B, C, H, W)
    return yfull.astype(np.float32), res


def kernel(x, wq, wk, wv, wo):
    out, _ = run(x, wq, wk, wv, wo, trace=False)
    return out
